# revision 15
# baseline (speedup 1.0000x reference)
"""Trainium2 Bass kernel for nn_BaseTimeAttention (dense transformer block:
QKV projection + RoPE + softmax attention + output projection).

Problem (hardcoded):
  x:  [B=2, S=2048, H=2048] fp32,  Wq/Wk/Wv/Wo: [2048, 2048] fp32
  out = softmax((rope(xWq^T) rope(xWk^T)^T)/sqrt(128)) (xWv^T) Wo^T

Sharding (8 cores): tensor-parallel over heads x data-parallel over batch.
Core c handles batch b=c//4 and head group g=c%4 (4 of 16 heads = 512 of 2048
channels). Each core produces a full [2048, 2048] partial of the output
projection restricted to its 512 input channels; the host sums 4 partials per
batch (o_proj row-parallel reduction on host).

The PE on TRN2 streams one output column per cycle regardless of operand
dtype (measured: bf16 216ns, fp32r 227ns per 128x128x512 matmul; fp8
DoubleRow also 216ns -- it doubles K per matmul but not column rate, so
hi/lo-compensated fp8 is a net loss).  All matmul streams are therefore
bf16, PSUM accumulates fp32, and Q^T/K^T/V stay SBUF-resident between phases
(no DRAM round trip).

RoPE with one output rounding: the [d,s] projection stays in PSUM (fp32), is
copied unrounded to an fp32r tile whose half-swap partner is produced by a
one-matmul multiply with a host permutation matrix (227ns on PE, replacing
SBUF->SBUF swap DMAs), and VectorE combines ps*cos + psw*sin in fp32 (fp32
tables) with a single final round into the resident bf16 q/k.  The P-matmul
of chain i is emitted during chain i+1 so the in-order PE queue never waits
on the ScalarE copy it depends on.

Phase 2 per head h / query block n (scoresT layout, no transposes):
    scoresT[s_k, s_q] = K-tile^T @ Q^T      (16 MMs; pairs share a 2-bank
                                             PSUM tile, pool bufs=3 so the
                                             next iteration never waits on
                                             the boundary exps)
    eT = exp(scoresT / sqrt(128))           (ScalarE, one ACTIVATE per pair,
                                             bf16 out)
    num[d, s_q] += V-tile^T @ eT            (PE, accumulate over s_k)
    den[:, s_q] += ones^T @ eT              (PE, broadcast row-sum; den rides
                                             otherwise-idle PE time since
                                             ScalarE exp is the co-bottleneck)
    yT = num * recip(den)                   (DVE; num/den PSUM is single-
                                             buffered -- PIPE=5 gives the
                                             reciprocal chain time before the
                                             next iteration's first
                                             accumulating matmul)

Phase 3: out[m, :] = yT^T @ Wo^T (4 accumulating MMs per [128,512] tile),
bf16 partials staged into [128,2048] rows, stores split across the sync and
scalar rings; host sums partials in fp32.

DMA: every tensor chunk is split into two column halves, one per HWDGE ring
(sync + scalar), issued in deadline order -- both rings carry identical byte
schedules so each chunk lands at the aggregate-bandwidth prefix time.  The
gpsimd SWDGE ring (~6x slower) only carries the tiny constants.
"""

import numpy as np
import ml_dtypes

import concourse.mybir as mybir
import concourse.tile as tile
from concourse import bacc
from concourse.bass_utils import run_bass_kernel_spmd

F32 = mybir.dt.float32
F32R = mybir.dt.float32r
BF16 = mybir.dt.bfloat16
AF = mybir.ActivationFunctionType
NPBF16 = ml_dtypes.bfloat16

B = 2
S = 2048
HIDDEN = 2048
HEADS = 16
DH = 128
THETA = 10000.0
N_CORES = 8
GROUPS = 4
HPC = HEADS // GROUPS  # heads per core
JPC = HPC * DH  # projection cols per core
SCALE = 1.0 / np.sqrt(DH)

SB = 512
NSB = S // SB
KT = HIDDEN // 128  # 16 contraction tiles
NKT = S // 128  # 16 s_k tiles

# weight/x chunk split: finest first so the first matmul starts early;
# chunks alternate between the sync and scalar HWDGE rings so both rings
# carry identical byte schedules and every chunk keeps 128 fat descriptors
CHS = (1, 1, 3, 3, 4, 4)  # chunk sizes (k-tiles)
CHO = (0, 1, 2, 5, 8, 12)  # chunk offsets
NCH = len(CHS)


def build():
    nc = bacc.Bacc("TRN2", target_bir_lowering=False, debug=False)

    x_d = nc.dram_tensor("xPM", [NSB, 128, KT, SB], BF16, kind="ExternalInput")
    wq_d = nc.dram_tensor("wqPM", [128, KT, JPC], BF16, kind="ExternalInput")
    wk_d = nc.dram_tensor("wkPM", [128, KT, JPC], BF16, kind="ExternalInput")
    wv_d = nc.dram_tensor("wvPM", [128, KT, JPC], BF16, kind="ExternalInput")
    wo_d = nc.dram_tensor("woPM", [128, HPC, HIDDEN], BF16, kind="ExternalInput")
    cos_d = nc.dram_tensor("cos", [DH, S], F32R, kind="ExternalInput")
    sin_d = nc.dram_tensor("sinS", [DH, S], F32R, kind="ExternalInput")
    ones_d = nc.dram_tensor("ones", [128, 128], BF16, kind="ExternalInput")
    perm_d = nc.dram_tensor("perm", [128, 128], F32R, kind="ExternalInput")
    out_d = nc.dram_tensor("out", [S, HIDDEN], BF16, kind="ExternalOutput")

    out = out_d.ap()

    def wslice(chunks, k, cols):
        for c in range(NCH):
            if k < CHO[c] + CHS[c]:
                return chunks[c][:, k - CHO[c], cols]
        raise AssertionError

    with tile.TileContext(nc) as tc:
        with tc.tile_pool(name="persist", bufs=1) as persist:
            ones_sb = persist.tile([128, 128], BF16, tag="ones")
            perm_sb = persist.tile([128, 128], F32R, tag="perm")
            nc.gpsimd.dma_start(out=ones_sb[:], in_=ones_d.ap())
            nc.gpsimd.dma_start(out=perm_sb[:], in_=perm_d.ap())

            # SBUF-resident projection outputs (bf16)
            qsb = [
                persist.tile([128, S], BF16, tag=f"qsb{j}", name=f"qsb{j}")
                for j in range(HPC)
            ]
            ksb = [
                persist.tile([128, S], BF16, tag=f"ksb{j}", name=f"ksb{j}")
                for j in range(HPC)
            ]
            vsb = [
                persist.tile([128, SB], BF16, tag=f"vsb{t}", name=f"vsb{t}")
                for t in range(NKT)
            ]

            # ---------------- Phase 1: projections + RoPE ------------------
            with (
                tc.tile_pool(name="p1w", bufs=1) as p1w,
                tc.tile_pool(name="p1x", bufs=2) as p1x,
                tc.tile_pool(name="p1cs", bufs=1) as p1cs,
                tc.tile_pool(name="p1s", bufs=3) as p1s,
                tc.tile_pool(name="p1ps", bufs=3, space="PSUM") as p1ps,
                tc.tile_pool(name="p1sw", bufs=2, space="PSUM") as p1sw,
            ):
                cosf = p1cs.tile([128, S], F32R, tag="cosf")
                sinf = p1cs.tile([128, S], F32R, tag="sinf")

                def ring(c):
                    return nc.sync if c % 2 == 0 else nc.scalar

                wch = {"q": [], "k": [], "v": []}

                def load_w_chunk(name, w_d, c):
                    w = p1w.tile([128, CHS[c], JPC], BF16, tag=f"w{name}{c}")
                    ring(c).dma_start(
                        out=w[:], in_=w_d.ap()[:, CHO[c] : CHO[c] + CHS[c], :]
                    )
                    wch[name].append(w)

                def load_x(s):
                    # two fat chunks, one per ring
                    xsc = []
                    for c, (o, sz) in enumerate(((0, 8), (8, 8))):
                        xt = p1x.tile([128, 8, SB], BF16, tag=f"xb{c}")
                        ring(c).dma_start(
                            out=xt[:], in_=x_d.ap()[s, :, o : o + sz, :]
                        )
                        xsc.append(xt)
                    return xsc

                def load_cs(s):
                    sblk = slice(s * SB, (s + 1) * SB)
                    nc.sync.dma_start(out=cosf[:, sblk], in_=cos_d.ap()[:, sblk])
                    nc.scalar.dma_start(
                        out=sinf[:, sblk], in_=sin_d.ap()[:, sblk]
                    )

                def xslice(xsc, k, cols):
                    return xsc[k // 8][:, k % 8, cols]

                # deadline-ordered issue: wq/x0 interleaved, then tables for
                # s=0, then wk, wv; later x blocks and tables are issued from
                # inside the s-loop so they don't clog the scalar queue
                xblocks = [None] * NSB
                x0 = []
                for c in range(NCH):
                    load_w_chunk("q", wq_d, c)
                    xt = p1x.tile([128, CHS[c], SB], BF16, tag=f"xs{c}")
                    ring(c).dma_start(
                        out=xt[:], in_=x_d.ap()[0, :, CHO[c] : CHO[c] + CHS[c], :]
                    )
                    x0.append(xt)
                load_cs(0)
                for c in range(NCH):
                    load_w_chunk("k", wk_d, c)
                for c in range(NCH):
                    load_w_chunk("v", wv_d, c)

                # pending rope work from the previous q/k chain (P-matmul
                # deferred one chain so the in-order PE never waits on the
                # ScalarE copy it depends on)
                pending = []

                def flush_pending():
                    for ps, qt, dst, sblk in pending:
                        psw = p1sw.tile([128, SB], F32, tag="psw")
                        nc.tensor.matmul(
                            psw[:], perm_sb[:], qt[:], start=True, stop=True
                        )
                        t1 = p1s.tile([128, SB], F32, tag="t1")
                        t2 = p1s.tile([128, SB], F32, tag="t2")
                        nc.vector.tensor_mul(t1[:], ps[:], cosf[:, sblk])
                        nc.vector.tensor_mul(t2[:], psw[:], sinf[:, sblk])
                        nc.vector.tensor_add(dst[:, sblk], t1[:], t2[:])
                    pending.clear()

                for s in range(NSB):
                    sblk = slice(s * SB, (s + 1) * SB)
                    xsc = x0 if s == 0 else xblocks[s]
                    xsl = wslice if s == 0 else xslice
                    first_chain = True
                    for name in ("q", "k", "v"):
                        for j in range(HPC):
                            jblk = slice(j * 128, (j + 1) * 128)
                            ps = p1ps.tile([128, SB], F32, tag="ps")
                            for k in range(KT):
                                if name != "v":  # Q/K: [j, s] transposed
                                    lhsT = wslice(wch[name], k, jblk)
                                    rhs = xsl(xsc, k, slice(0, SB))
                                else:  # V: natural [s, j]
                                    lhsT = xsl(xsc, k, jblk)
                                    rhs = wslice(wch[name], k, slice(0, JPC))
                                nc.tensor.matmul(
                                    ps[:],
                                    lhsT,
                                    rhs,
                                    start=(k == 0),
                                    stop=(k == KT - 1),
                                )
                            flush_pending()
                            if name != "v":
                                qt = p1s.tile([128, SB], F32R, tag="qt")
                                nc.scalar.copy(qt[:], ps[:])
                                dst = qsb[j] if name == "q" else ksb[j]
                                pending.append((ps, qt, dst, sblk))
                            else:
                                nc.scalar.copy(vsb[s * HPC + j][:], ps[:])
                            if first_chain:
                                first_chain = False
                                if s + 1 < NSB:
                                    xblocks[s + 1] = load_x(s + 1)
                                    load_cs(s + 1)
                    flush_pending()

            # ---------------- Phases 2+3 -----------------------------------
            with tc.tile_pool(name="p23", bufs=1) as p23:
                yt = p23.tile([128, HPC, S], BF16, tag="yt")
                wo = p23.tile([128, HPC, HIDDEN], BF16, tag="wo")
                for kj in range(HPC):
                    ring = nc.sync if kj < 2 else nc.scalar
                    ring.dma_start(out=wo[:, kj, :], in_=wo_d.ap()[:, kj, :])

                # ---------------- Phase 2: attention -----------------------
                with (
                    tc.tile_pool(name="p2e", bufs=6) as p2e,
                    tc.tile_pool(name="p2r", bufs=2) as p2r,
                    tc.tile_pool(name="p2sc", bufs=3, space="PSUM") as p2sc,
                    tc.tile_pool(name="p2nd", bufs=1, space="PSUM") as p2nd,
                ):
                    for h in range(HPC):
                        qh, kh = qsb[h], ksb[h]
                        for n in range(NSB):
                            nblk = slice(n * SB, (n + 1) * SB)
                            num = p2nd.tile([128, SB], F32, tag="num")
                            den = p2nd.tile([128, SB], F32, tag="den")
                            DLAG = 2  # den trails scores by 2 pairs
                            NLAG = 5  # num trails by 5: the reciprocal of
                            # den (ready early) and the num*r multiply both
                            # clear before the next iteration's first
                            # accumulating matmul reuses the 1-buf PSUM
                            NP = NKT // 2  # 8 score pairs
                            es = [None] * NP
                            for p in range(NP + NLAG):
                                if p < NP:
                                    sc2 = p2sc.tile([128, 2, SB], F32, tag="sc")
                                    e2 = p2e.tile([128, 2, SB], BF16, tag="e")
                                    for half in range(2):
                                        i = 2 * p + half
                                        nc.tensor.matmul(
                                            sc2[:, half, :],
                                            kh[:, i * 128 : (i + 1) * 128],
                                            qh[:, nblk],
                                            start=True,
                                            stop=True,
                                        )
                                    nc.scalar.activation(
                                        e2[:], sc2[:], AF.Exp, scale=float(SCALE)
                                    )
                                    es[p] = e2
                                if DLAG <= p < NP + DLAG:
                                    pp = p - DLAG
                                    for half in range(2):
                                        i = 2 * pp + half
                                        nc.tensor.matmul(
                                            den[:],
                                            ones_sb[:],
                                            es[pp][:, half, :],
                                            start=(i == 0),
                                            stop=(i == NKT - 1),
                                        )
                                if p >= NLAG:
                                    pp = p - NLAG
                                    for half in range(2):
                                        i = 2 * pp + half
                                        nc.tensor.matmul(
                                            num[:],
                                            vsb[i][:, h * 128 : (h + 1) * 128],
                                            es[pp][:, half, :],
                                            start=(i == 0),
                                            stop=(i == NKT - 1),
                                        )
                            r = p2r.tile([128, SB], F32, tag="r")
                            scr = p2r.tile([128, SB], F32, tag="scr")
                            nc.vector.reciprocal_approx_accurate(
                                out=r[:], in_=den[:], scratch=scr[:]
                            )
                            nc.vector.tensor_mul(yt[:, h, nblk], num[:], r[:])

                # ------------ Phase 3: output projection -------------------
                with (
                    tc.tile_pool(name="p3s", bufs=2) as p3s,
                    tc.tile_pool(name="p3ps", bufs=4, space="PSUM") as p3ps,
                ):
                    for m in range(S // 128):
                        mblk = slice(m * 128, (m + 1) * 128)
                        oc = p3s.tile([128, HIDDEN], BF16, tag="oc")
                        for n in range(HIDDEN // SB):
                            nblk = slice(n * SB, (n + 1) * SB)
                            ps = p3ps.tile([128, SB], F32, tag="ps")
                            for kj in range(HPC):
                                nc.tensor.matmul(
                                    ps[:],
                                    yt[:, kj, mblk],
                                    wo[:, kj, nblk],
                                    start=(kj == 0),
                                    stop=(kj == HPC - 1),
                                )
                            nc.scalar.copy(oc[:, nblk], ps[:])
                        ring = nc.sync if m % 2 == 0 else nc.scalar
                        ring.dma_start(out=out[mblk, :], in_=oc[:])

    nc.compile()
    return nc


_NC = None


def _get_nc():
    global _NC
    if _NC is None:
        _NC = build()
    return _NC


def _rope_tables():
    inv_freq = 1.0 / (THETA ** (np.arange(0, DH, 2, dtype=np.float32) / DH))
    freqs = np.arange(S, dtype=np.float32)[:, None] * inv_freq[None, :]  # [S, 64]
    cos_h = np.cos(freqs).T.astype(np.float32)  # [64, S]
    sin_h = np.sin(freqs).T.astype(np.float32)
    cos = np.concatenate([cos_h, cos_h], axis=0)  # [128, S]
    sin_s = np.concatenate([-sin_h, sin_h], axis=0)  # [128, S]
    return cos, sin_s


def _bf(a):
    return np.ascontiguousarray(a.astype(NPBF16))


def _pm_weight(wT):  # [2048, 512] (k, j) -> [128, 16, 512] partition-major
    return _bf(wT.reshape(KT, 128, JPC).transpose(1, 0, 2))


def _make_in_maps(inputs):
    x = np.asarray(inputs["x"], dtype=np.float32)
    Wq = np.asarray(inputs["Wq"], dtype=np.float32)
    Wk = np.asarray(inputs["Wk"], dtype=np.float32)
    Wv = np.asarray(inputs["Wv"], dtype=np.float32)
    Wo = np.asarray(inputs["Wo"], dtype=np.float32)

    cos, sin_s = _rope_tables()
    ones = np.ones((128, 128), dtype=np.float32)
    # half-swap permutation: perm @ v swaps the two 64-row halves
    perm = np.zeros((128, 128), dtype=np.float32)
    perm[np.arange(64), np.arange(64, 128)] = 1.0
    perm[np.arange(64, 128), np.arange(64)] = 1.0

    in_maps = []
    for c in range(N_CORES):
        b = c // GROUPS
        g = c % GROUPS
        rows = slice(g * JPC, (g + 1) * JPC)
        xT = x[b].T  # [hidden(k), s]
        # [k, s] -> [s_blk, p, kt, s_in_blk]
        xpm = _bf(xT.reshape(KT, 128, NSB, SB).transpose(2, 1, 0, 3))
        # Wo[:, rows].T -> [512(j), 2048] -> [p, kj, 2048]
        woT = Wo[:, rows].T
        wopm = _bf(woT.reshape(HPC, 128, HIDDEN).transpose(1, 0, 2))
        in_maps.append(
            {
                "xPM": xpm,
                "wqPM": _pm_weight(Wq[rows].T),
                "wkPM": _pm_weight(Wk[rows].T),
                "wvPM": _pm_weight(Wv[rows].T),
                "woPM": wopm,
                "cos": np.ascontiguousarray(cos),
                "sinS": np.ascontiguousarray(sin_s),
                "ones": _bf(ones),
                "perm": np.ascontiguousarray(perm),
            }
        )
    return in_maps


def kernel(x, Wq, Wk, Wv, Wo):
    nc = _get_nc()
    in_maps = _make_in_maps({"x": x, "Wq": Wq, "Wk": Wk, "Wv": Wv, "Wo": Wo})
    res = run_bass_kernel_spmd(nc, in_maps, list(range(N_CORES)))

    out = np.zeros((B, S, HIDDEN), dtype=np.float32)
    for c in range(N_CORES):
        out[c // GROUPS] += np.asarray(res.results[c]["out"]).astype(np.float32)
    return out


# revision 16
# speedup vs baseline: 1.1965x; 1.1965x over previous
"""Trainium2 Bass kernel for nn_BaseTimeAttention (dense transformer block:
QKV projection + RoPE + softmax attention + output projection).

Problem (hardcoded):
  x:  [B=2, S=2048, H=2048] fp32,  Wq/Wk/Wv/Wo: [2048, 2048] fp32
  out = softmax((rope(xWq^T) rope(xWk^T)^T)/sqrt(128)) (xWv^T) Wo^T

Sharding (8 cores): tensor-parallel over heads x data-parallel over batch.
Core c handles batch b=c//4 and head group g=c%4 (4 of 16 heads = 512 of 2048
channels). Each core produces a full [2048, 2048] partial of the output
projection restricted to its 512 input channels; the host sums 4 partials per
batch (o_proj row-parallel reduction on host).

The PE on TRN2 streams one output column per cycle regardless of operand
dtype (measured: bf16 216ns, fp32r 227ns per 128x128x512 matmul; fp8
DoubleRow also 216ns -- it doubles K per matmul but not column rate, so
hi/lo-compensated fp8 is a net loss).  All matmul streams are therefore
bf16, PSUM accumulates fp32, and Q^T/K^T/V stay SBUF-resident between phases
(no DRAM round trip).

RoPE with one output rounding: the [d,s] projection stays in PSUM (fp32), is
copied unrounded to an fp32r tile whose half-swap partner is produced by a
one-matmul multiply with a host permutation matrix (227ns on PE, replacing
SBUF->SBUF swap DMAs), and VectorE combines ps*cos + psw*sin in fp32 (fp32
tables) with a single final round into the resident bf16 q/k.  The P-matmul
of chain i is emitted during chain i+1 so the in-order PE queue never waits
on the ScalarE copy it depends on.

Phase 2 per head h / query block n (scoresT layout, no transposes):
    scoresT[s_k, s_q] = K-tile^T @ Q^T      (16 MMs; pairs share a 2-bank
                                             PSUM tile, pool bufs=3 so the
                                             next iteration never waits on
                                             the boundary exps)
    eT = exp(scoresT / sqrt(128))           (ScalarE, one ACTIVATE per pair,
                                             bf16 out)
    num[d, s_q] += V-tile^T @ eT            (PE, accumulate over s_k)
    den[:, s_q] += ones^T @ eT              (PE, broadcast row-sum; den rides
                                             otherwise-idle PE time since
                                             ScalarE exp is the co-bottleneck)
    yT = num * recip(den)                   (DVE; num/den PSUM is single-
                                             buffered -- PIPE=5 gives the
                                             reciprocal chain time before the
                                             next iteration's first
                                             accumulating matmul)

Phase 3: out[m, :] = yT^T @ Wo^T (4 accumulating MMs per [128,512] tile),
bf16 partials staged into [128,2048] rows, stores split across the sync and
scalar rings; host sums partials in fp32.

DMA: every tensor chunk is split into two column halves, one per HWDGE ring
(sync + scalar), issued in deadline order -- both rings carry identical byte
schedules so each chunk lands at the aggregate-bandwidth prefix time.  The
gpsimd SWDGE ring (~6x slower) only carries the tiny constants.
"""

import numpy as np
import ml_dtypes

import concourse.mybir as mybir
import concourse.tile as tile
from concourse import bacc
from concourse.bass_utils import run_bass_kernel_spmd

F32 = mybir.dt.float32
F32R = mybir.dt.float32r
BF16 = mybir.dt.bfloat16
AF = mybir.ActivationFunctionType
NPBF16 = ml_dtypes.bfloat16

B = 2
S = 2048
HIDDEN = 2048
HEADS = 16
DH = 128
THETA = 10000.0
N_CORES = 8
GROUPS = 4
HPC = HEADS // GROUPS  # heads per core
JPC = HPC * DH  # projection cols per core
SCALE = 1.0 / np.sqrt(DH)

SB = 512
NSB = S // SB
KT = HIDDEN // 128  # 16 contraction tiles
NKT = S // 128  # 16 s_k tiles

# weight/x chunk split: finest first so the first matmul starts early;
# chunks alternate between the sync and scalar HWDGE rings so both rings
# carry identical byte schedules and every chunk keeps 128 fat descriptors
CHS = (1, 1, 3, 3, 4, 4)  # chunk sizes (k-tiles)
CHO = (0, 1, 2, 5, 8, 12)  # chunk offsets
NCH = len(CHS)


def build():
    nc = bacc.Bacc("TRN2", target_bir_lowering=False, debug=False)

    x_d = nc.dram_tensor("xPM", [NSB, 128, KT, SB], BF16, kind="ExternalInput")
    wq_d = nc.dram_tensor("wqPM", [128, KT, JPC], BF16, kind="ExternalInput")
    wk_d = nc.dram_tensor("wkPM", [128, KT, JPC], BF16, kind="ExternalInput")
    wv_d = nc.dram_tensor("wvPM", [128, KT, JPC], BF16, kind="ExternalInput")
    wo_d = nc.dram_tensor("woPM", [128, HPC, HIDDEN], BF16, kind="ExternalInput")
    cos_d = nc.dram_tensor("cos", [DH, S], F32R, kind="ExternalInput")
    sin_d = nc.dram_tensor("sinS", [DH, S], F32R, kind="ExternalInput")
    ones_d = nc.dram_tensor("ones", [128, 128], BF16, kind="ExternalInput")
    perm_d = nc.dram_tensor("perm", [128, 128], F32R, kind="ExternalInput")
    out_d = nc.dram_tensor("out", [S, HIDDEN], BF16, kind="ExternalOutput")

    out = out_d.ap()

    def wslice(chunks, k, cols):
        for c in range(NCH):
            if k < CHO[c] + CHS[c]:
                return chunks[c][:, k - CHO[c], cols]
        raise AssertionError

    with tile.TileContext(nc) as tc:
        with tc.tile_pool(name="persist", bufs=1) as persist:
            ones_sb = persist.tile([128, 128], BF16, tag="ones")
            perm_sb = persist.tile([128, 128], F32R, tag="perm")
            nc.gpsimd.dma_start(out=ones_sb[:], in_=ones_d.ap())
            nc.gpsimd.dma_start(out=perm_sb[:], in_=perm_d.ap())

            # SBUF-resident projection outputs (bf16)
            qsb = [
                persist.tile([128, S], BF16, tag=f"qsb{j}", name=f"qsb{j}")
                for j in range(HPC)
            ]
            ksb = [
                persist.tile([128, S], BF16, tag=f"ksb{j}", name=f"ksb{j}")
                for j in range(HPC)
            ]
            vsb = [
                persist.tile([128, SB], BF16, tag=f"vsb{t}", name=f"vsb{t}")
                for t in range(NKT)
            ]

            # ---------------- Phase 1: projections + RoPE ------------------
            with (
                tc.tile_pool(name="p1w", bufs=1) as p1w,
                tc.tile_pool(name="p1x", bufs=2) as p1x,
                tc.tile_pool(name="p1cs", bufs=1) as p1cs,
                tc.tile_pool(name="p1s", bufs=3) as p1s,
                tc.tile_pool(name="p1ps", bufs=3, space="PSUM") as p1ps,
                tc.tile_pool(name="p1sw", bufs=2, space="PSUM") as p1sw,
            ):
                cosf = p1cs.tile([128, S], F32R, tag="cosf")
                sinf = p1cs.tile([128, S], F32R, tag="sinf")

                def ring(c):
                    return nc.sync if c % 2 == 0 else nc.scalar

                wch = {"q": [], "k": [], "v": []}

                def load_w_chunk(name, w_d, c):
                    w = p1w.tile([128, CHS[c], JPC], BF16, tag=f"w{name}{c}")
                    ring(c).dma_start(
                        out=w[:], in_=w_d.ap()[:, CHO[c] : CHO[c] + CHS[c], :]
                    )
                    wch[name].append(w)

                def load_x(s):
                    # two fat chunks, one per ring
                    xsc = []
                    for c, (o, sz) in enumerate(((0, 8), (8, 8))):
                        xt = p1x.tile([128, 8, SB], BF16, tag=f"xb{c}")
                        ring(c).dma_start(
                            out=xt[:], in_=x_d.ap()[s, :, o : o + sz, :]
                        )
                        xsc.append(xt)
                    return xsc

                def load_cs(s):
                    sblk = slice(s * SB, (s + 1) * SB)
                    nc.sync.dma_start(out=cosf[:, sblk], in_=cos_d.ap()[:, sblk])
                    nc.scalar.dma_start(
                        out=sinf[:, sblk], in_=sin_d.ap()[:, sblk]
                    )

                def xslice(xsc, k, cols):
                    return xsc[k // 8][:, k % 8, cols]

                # deadline-ordered issue: wq/x0 interleaved, then tables for
                # s=0, then wk, wv; later x blocks and tables are issued from
                # inside the s-loop so they don't clog the scalar queue
                xblocks = [None] * NSB
                x0 = []
                for c in range(NCH):
                    load_w_chunk("q", wq_d, c)
                    xt = p1x.tile([128, CHS[c], SB], BF16, tag=f"xs{c}")
                    ring(c).dma_start(
                        out=xt[:], in_=x_d.ap()[0, :, CHO[c] : CHO[c] + CHS[c], :]
                    )
                    x0.append(xt)
                load_cs(0)
                for c in range(NCH):
                    load_w_chunk("k", wk_d, c)
                for c in range(NCH):
                    load_w_chunk("v", wv_d, c)

                # pending rope work from the previous q/k chain (P-matmul
                # deferred one chain so the in-order PE never waits on the
                # ScalarE copy it depends on)
                pending = []

                def flush_pending():
                    for ps, qt, dst, sblk in pending:
                        psw = p1sw.tile([128, SB], F32, tag="psw")
                        nc.tensor.matmul(
                            psw[:], perm_sb[:], qt[:], start=True, stop=True
                        )
                        t1 = p1s.tile([128, SB], F32, tag="t1")
                        t2 = p1s.tile([128, SB], F32, tag="t2")
                        nc.vector.tensor_mul(t1[:], ps[:], cosf[:, sblk])
                        nc.vector.tensor_mul(t2[:], psw[:], sinf[:, sblk])
                        nc.vector.tensor_add(dst[:, sblk], t1[:], t2[:])
                    pending.clear()

                for s in range(NSB):
                    sblk = slice(s * SB, (s + 1) * SB)
                    xsc = x0 if s == 0 else xblocks[s]
                    xsl = wslice if s == 0 else xslice
                    first_chain = True
                    for name in ("q", "k", "v"):
                        for j in range(HPC):
                            jblk = slice(j * 128, (j + 1) * 128)
                            ps = p1ps.tile([128, SB], F32, tag="ps")
                            for k in range(KT):
                                if name != "v":  # Q/K: [j, s] transposed
                                    lhsT = wslice(wch[name], k, jblk)
                                    rhs = xsl(xsc, k, slice(0, SB))
                                else:  # V: natural [s, j]
                                    lhsT = xsl(xsc, k, jblk)
                                    rhs = wslice(wch[name], k, slice(0, JPC))
                                nc.tensor.matmul(
                                    ps[:],
                                    lhsT,
                                    rhs,
                                    start=(k == 0),
                                    stop=(k == KT - 1),
                                )
                            flush_pending()
                            if name != "v":
                                qt = p1s.tile([128, SB], F32R, tag="qt")
                                nc.scalar.copy(qt[:], ps[:])
                                dst = qsb[j] if name == "q" else ksb[j]
                                pending.append((ps, qt, dst, sblk))
                            else:
                                nc.scalar.copy(vsb[s * HPC + j][:], ps[:])
                            if first_chain:
                                first_chain = False
                                if s + 1 < NSB:
                                    xblocks[s + 1] = load_x(s + 1)
                                    load_cs(s + 1)
                    flush_pending()

            # ---------------- Phases 2+3 -----------------------------------
            with tc.tile_pool(name="p23", bufs=1) as p23:
                yt = p23.tile([128, HPC, S], BF16, tag="yt")
                wo = p23.tile([128, HPC, HIDDEN], BF16, tag="wo")
                for kj in range(HPC):
                    ring = nc.sync if kj < 2 else nc.scalar
                    ring.dma_start(out=wo[:, kj, :], in_=wo_d.ap()[:, kj, :])

                # ---------------- Phase 2: attention -----------------------
                with (
                    tc.tile_pool(name="p2e", bufs=7) as p2e,
                    tc.tile_pool(name="p2r", bufs=2) as p2r,
                    tc.tile_pool(name="p2sc", bufs=3, space="PSUM") as p2sc,
                    tc.tile_pool(name="p2nd", bufs=1, space="PSUM") as p2nd,
                ):
                    for h in range(HPC):
                        qh, kh = qsb[h], ksb[h]
                        for n in range(NSB):
                            nblk = slice(n * SB, (n + 1) * SB)
                            num = p2nd.tile([128, SB], F32, tag="num")
                            den = p2nd.tile([128, SB], F32, tag="den")
                            DLAG = 4  # den trails scores by 4 pairs
                            # (enough cover for the ~1.4us exp latency)
                            NLAG = 6  # num trails by 6: the reciprocal of
                            # den (ready early) and the num*r multiply both
                            # clear before the next iteration's first
                            # accumulating matmul reuses the 1-buf PSUM
                            NP = NKT // 2  # 8 score pairs
                            es = [None] * NP
                            for p in range(NP + NLAG):
                                if p < NP:
                                    sc2 = p2sc.tile([128, 2, SB], F32, tag="sc")
                                    e2 = p2e.tile([128, 2, SB], BF16, tag="e")
                                    for half in range(2):
                                        i = 2 * p + half
                                        nc.tensor.matmul(
                                            sc2[:, half, :],
                                            kh[:, i * 128 : (i + 1) * 128],
                                            qh[:, nblk],
                                            start=True,
                                            stop=True,
                                        )
                                    nc.scalar.activation(
                                        e2[:], sc2[:], AF.Exp, scale=float(SCALE)
                                    )
                                    es[p] = e2
                                if DLAG <= p < NP + DLAG:
                                    pp = p - DLAG
                                    for half in range(2):
                                        i = 2 * pp + half
                                        nc.tensor.matmul(
                                            den[:],
                                            ones_sb[:],
                                            es[pp][:, half, :],
                                            start=(i == 0),
                                            stop=(i == NKT - 1),
                                        )
                                if p >= NLAG:
                                    pp = p - NLAG
                                    for half in range(2):
                                        i = 2 * pp + half
                                        nc.tensor.matmul(
                                            num[:],
                                            vsb[i][:, h * 128 : (h + 1) * 128],
                                            es[pp][:, half, :],
                                            start=(i == 0),
                                            stop=(i == NKT - 1),
                                        )
                            r = p2r.tile([128, SB], F32, tag="r")
                            scr = p2r.tile([128, SB], F32, tag="scr")
                            nc.vector.reciprocal_approx_accurate(
                                out=r[:], in_=den[:], scratch=scr[:]
                            )
                            nc.vector.tensor_mul(yt[:, h, nblk], num[:], r[:])

                # ------------ Phase 3: output projection -------------------
                with (
                    tc.tile_pool(name="p3s", bufs=2) as p3s,
                    tc.tile_pool(name="p3ps", bufs=4, space="PSUM") as p3ps,
                ):
                    for m in range(S // 128):
                        mblk = slice(m * 128, (m + 1) * 128)
                        oc = p3s.tile([128, HIDDEN], BF16, tag="oc")
                        for n in range(HIDDEN // SB):
                            nblk = slice(n * SB, (n + 1) * SB)
                            ps = p3ps.tile([128, SB], F32, tag="ps")
                            for kj in range(HPC):
                                nc.tensor.matmul(
                                    ps[:],
                                    yt[:, kj, mblk],
                                    wo[:, kj, nblk],
                                    start=(kj == 0),
                                    stop=(kj == HPC - 1),
                                )
                            nc.scalar.copy(oc[:, nblk], ps[:])
                        ring = nc.sync if m % 2 == 0 else nc.scalar
                        ring.dma_start(out=out[mblk, :], in_=oc[:])

    nc.compile()
    return nc


_NC = None


def _get_nc():
    global _NC
    if _NC is None:
        _NC = build()
    return _NC


def _rope_tables():
    inv_freq = 1.0 / (THETA ** (np.arange(0, DH, 2, dtype=np.float32) / DH))
    freqs = np.arange(S, dtype=np.float32)[:, None] * inv_freq[None, :]  # [S, 64]
    cos_h = np.cos(freqs).T.astype(np.float32)  # [64, S]
    sin_h = np.sin(freqs).T.astype(np.float32)
    cos = np.concatenate([cos_h, cos_h], axis=0)  # [128, S]
    sin_s = np.concatenate([-sin_h, sin_h], axis=0)  # [128, S]
    return cos, sin_s


def _bf(a):
    return np.ascontiguousarray(a.astype(NPBF16))


def _pm_weight(wT):  # [2048, 512] (k, j) -> [128, 16, 512] partition-major
    return _bf(wT.reshape(KT, 128, JPC).transpose(1, 0, 2))


def _make_in_maps(inputs):
    x = np.asarray(inputs["x"], dtype=np.float32)
    Wq = np.asarray(inputs["Wq"], dtype=np.float32)
    Wk = np.asarray(inputs["Wk"], dtype=np.float32)
    Wv = np.asarray(inputs["Wv"], dtype=np.float32)
    Wo = np.asarray(inputs["Wo"], dtype=np.float32)

    cos, sin_s = _rope_tables()
    ones = np.ones((128, 128), dtype=np.float32)
    # half-swap permutation: perm @ v swaps the two 64-row halves
    perm = np.zeros((128, 128), dtype=np.float32)
    perm[np.arange(64), np.arange(64, 128)] = 1.0
    perm[np.arange(64, 128), np.arange(64)] = 1.0

    in_maps = []
    for c in range(N_CORES):
        b = c // GROUPS
        g = c % GROUPS
        rows = slice(g * JPC, (g + 1) * JPC)
        xT = x[b].T  # [hidden(k), s]
        # [k, s] -> [s_blk, p, kt, s_in_blk]
        xpm = _bf(xT.reshape(KT, 128, NSB, SB).transpose(2, 1, 0, 3))
        # Wo[:, rows].T -> [512(j), 2048] -> [p, kj, 2048]
        woT = Wo[:, rows].T
        wopm = _bf(woT.reshape(HPC, 128, HIDDEN).transpose(1, 0, 2))
        in_maps.append(
            {
                "xPM": xpm,
                "wqPM": _pm_weight(Wq[rows].T),
                "wkPM": _pm_weight(Wk[rows].T),
                "wvPM": _pm_weight(Wv[rows].T),
                "woPM": wopm,
                "cos": np.ascontiguousarray(cos),
                "sinS": np.ascontiguousarray(sin_s),
                "ones": _bf(ones),
                "perm": np.ascontiguousarray(perm),
            }
        )
    return in_maps


def kernel(x, Wq, Wk, Wv, Wo):
    nc = _get_nc()
    in_maps = _make_in_maps({"x": x, "Wq": Wq, "Wk": Wk, "Wv": Wv, "Wo": Wo})
    res = run_bass_kernel_spmd(nc, in_maps, list(range(N_CORES)))

    out = np.zeros((B, S, HIDDEN), dtype=np.float32)
    for c in range(N_CORES):
        out[c // GROUPS] += np.asarray(res.results[c]["out"]).astype(np.float32)
    return out
